# revision 50
# baseline (speedup 1.0000x reference)
"""Trainium2 Bass kernel v6: batched causal attention (B=4, S=4096, E=256, f32).

Sharding: 2 cores per batch element; QUERY chunks split within the pair
(even core gets 512-row chunks {7,5,2,0}, odd {6,4,3,1}) so causal work is
perfectly balanced with NO cross-core communication.  Both cores hold full
K/V for their batch.  SPMD-uniform instruction stream: 4 slots with padded
k-tile counts (32,24,16,8); per-core DATA (mask table) kills padding tiles
and applies causal masks.

v6 key points:
  - all inputs shipped bf16; Z is shipped BOTH natural (PV stationary) and
    pre-TRANSPOSED (projection moving operand); X only pre-transposed.
    This removes all 96 PE transposes + their PSUM round-trips.
  - output written bf16, host upcasts.
  - PE warm-up matmuls ramp the clock while the first chunks stream in.
  - causal masking additive on the PE (-1e9 ident x blocked-pattern fp8
    moving), fused into the score accumulation group.
  - score/mask matmuls + exp skip fully-blocked column prefixes in the
    last-8 k-tile window (start = max(0,(w-4)*128), parity-uniform).

Per-core dataflow (bf16 matmuls, f32 PSUM):
  phase1: Q^T=(WqT@X^T + bq)/16, K^T=WkT@Z^T directly from DMA'd
  transposed chunks, interleaved into slot 0's attention pairs.
  attention: per k-tile pair, S^T = K^T(stat).Q^T (+mask matmul on diag);
  exp on scalar -> P^T bf16; O'^T += Z(stat)@P^T (V projection deferred);
  rowsum via DVE pair-sums + gpsimd accumulation (chain tail on DVE so the
  per-slot reduce unblocks sooner).
  post (per slot, overlapped): rowsum reduced by ones-matmul, reciprocal
  on DVE, O = O'@Wv^T + bv*rowsum (rank-1), scaled by 1/rowsum, bf16 out
  (last slot: per-128-row output DMAs to shorten the tail).

Measured on 8xTRN2: ~119.4us HW exec (baseline v3: 144-168us), rel err
6.2e-3 vs the f32 reference (gate 2e-2).
"""

import numpy as np

B = 4
S = 4096
E = 256
NSLOT = 4
PADN = (32, 24, 16, 8)       # padded k-tiles per slot
CHUNKS = ((7, 5, 2, 0), (6, 4, 3, 1))   # slot -> 512-chunk, per parity
NMASK = 4                    # masked pairs per slot (last 4)

_COMPILED = {}


def _build():
    import concourse.bass as bass
    import concourse.tile as tile
    from concourse import mybir, bacc
    from concourse.masks import make_identity

    f32 = mybir.dt.float32
    bf16 = mybir.dt.bfloat16
    fp8 = mybir.dt.float8e4
    Exp = mybir.ActivationFunctionType.Exp
    Add = mybir.AluOpType.add

    nc = bacc.Bacc("TRN2", target_bir_lowering=False, debug=False,
                   enable_asserts=True, num_devices=8)

    # all big inputs are pre-arranged host-side into partition-major layouts
    # so every DMA is identity-mapped: 128 descriptor rows, cheap to issue.
    z_ext = nc.dram_tensor("znat", [128, 32, E], bf16, kind="ExternalInput")
    zt_ext = nc.dram_tensor("zt", [128, 8, 2, 512], bf16,
                            kind="ExternalInput")
    xt_ext = nc.dram_tensor("xt", [128, 4, 2, 512], bf16,
                            kind="ExternalInput")
    w_ext = nc.dram_tensor("wall", [128, 3, 2, 256], bf16,
                           kind="ExternalInput")
    bqs_ext = nc.dram_tensor("bqs", [128, 2], f32, kind="ExternalInput")
    ebias_ext = nc.dram_tensor("ebias", [40], f32, kind="ExternalInput")
    bv_ext = nc.dram_tensor("bvr", [E], bf16, kind="ExternalInput")
    masks_ext = nc.dram_tensor("masks", [128, 16, 2, 512], fp8,
                               kind="ExternalInput")
    out_ext = nc.dram_tensor("out", [2048, E], bf16, kind="ExternalOutput")

    with tile.TileContext(nc) as tc:
        with tc.tile_pool(name="singles", bufs=1) as singles:
            ident_bf = singles.tile([128, 128], bf16)
            make_identity(nc, ident_bf[:])

            # ---- big persistent SBUF (per-chunk tiles so DMAs overlap) -----
            z_nat = [singles.tile([128, 4, E], bf16, name=f"z_nat{i}")
                     for i in range(8)]
            zT = [singles.tile([128, 2, 512], bf16, name=f"zT{i}")
                  for i in range(8)]
            xT = [singles.tile([128, 2, 512], bf16, name=f"xT{i}")
                  for i in range(4)]
            kT2 = [singles.tile([128, 2, 512], bf16, name=f"kT2_{i}")
                   for i in range(8)]
            qT2 = [singles.tile([128, 2, 512], bf16, name=f"qT2_{i}")
                   for i in range(4)]
            maskt = singles.tile([128, 16, 2, 512], fp8)
            w_all = singles.tile([128, 3, 2, 256], bf16, name="w_all")

            # ---- per-chunk identity-layout input DMAs, spread over the
            # three DMA issue paths (sync+scalar HWDGE, gpsimd SWDGE).
            # Rings are FIFO at ~1/3 of the 358GB/s core budget each, so
            # order strictly by first-use time; tiny tensors go first. ------
            bqs = singles.tile([128, 2], f32)
            nc.sync.dma_start(out=bqs[:], in_=bqs_ext[:])
            nc.scalar.dma_start(out=w_all[:], in_=w_ext.ap())
            ebias = singles.tile([128, 40], f32)
            nc.scalar.dma_start(
                out=ebias[:],
                in_=bass.AP(tensor=ebias_ext, offset=0, ap=[[0, 128], [1, 40]]))

            for _c in range(8):
                eng = nc.sync if _c % 2 == 0 else nc.gpsimd
                eng.dma_start(out=zT[_c][:], in_=zt_ext[:, _c, :, :])
                eng.dma_start(out=z_nat[_c][:],
                              in_=z_ext[:, 4 * _c:4 * (_c + 1), :])
                if _c < 4:
                    nc.scalar.dma_start(out=xT[_c][:],
                                        in_=xt_ext[:, _c, :, :])
            # masks ride at the tail of the sync/gpsimd rings: first needed
            # at pair 12 (gm<8) / pair 32 (gm>=8), long after the chunks.
            nc.sync.dma_start(out=maskt[:, 0:8, :, :],
                              in_=masks_ext[:, 0:8, :, :])
            nc.gpsimd.dma_start(out=maskt[:, 8:16, :, :],
                                in_=masks_ext[:, 8:16, :, :])

            # ---- PE warm-up: dummy matmuls ramp the clock to full p-state
            # while the first chunks stream in -------------------------------
            warm = singles.tile([128, 512], bf16)
            nc.vector.memset(warm[:], 0.0)
            with tc.tile_pool(name="ps_w", bufs=1, space="PSUM") as ps_w:
                psw = ps_w.tile([128, 512], f32)
                for _ in range(13):
                    nc.tensor.matmul(psw[:], warm[:, 0:128], warm[:],
                                     start=True, stop=True)

            ident_big = singles.tile([128, 128], bf16)
            nc.vector.tensor_scalar_mul(ident_big[:], ident_bf[:], -1e9)
            bv_sb = singles.tile([1, E], bf16)
            nc.scalar.dma_start(out=bv_sb[:], in_=bv_ext.ap().rearrange(
                "(one e) -> one e", one=1))
            ones_full = singles.tile([128, 128], f32)
            nc.vector.memset(ones_full[:], 1.0)

            with tc.tile_pool(name="ps_s", bufs=2, space="PSUM") as ps_s, \
                 tc.tile_pool(name="pTp", bufs=6) as pTp, \
                 tc.tile_pool(name="rsp", bufs=4) as rsp, \
                 tc.tile_pool(name="postp", bufs=2) as postp:

                # ---- phase 1: projections from pre-transposed chunks -------
                def kproj(sc):
                    psk = ps_s.tile([128, 2, 512], f32, tag="pss", name="psk")
                    for ft in range(2):
                        for eh in range(2):
                            nc.tensor.matmul(
                                psk[:, ft, :],
                                w_all[:, 1, eh, 128 * ft:128 * (ft + 1)],
                                zT[sc][:, eh, :],
                                start=(eh == 0), stop=(eh == 1))
                    nc.vector.tensor_copy(out=kT2[sc][:], in_=psk[:])

                def qproj(s):
                    psq = ps_s.tile([128, 2, 512], f32, tag="pss", name="psq")
                    for ft in range(2):
                        for eh in range(2):
                            nc.tensor.matmul(
                                psq[:, ft, :],
                                w_all[:, 0, eh, 128 * ft:128 * (ft + 1)],
                                xT[s][:, eh, :],
                                start=(eh == 0), stop=(eh == 1))
                    for ft in range(2):
                        nc.vector.tensor_scalar(
                            out=qT2[s][:, ft, :],
                            in0=psq[:, ft, :],
                            scalar1=1.0 / 16.0, scalar2=bqs[:, ft:ft + 1],
                            op0=mybir.AluOpType.mult, op1=Add)

                # ---- attention (phase 1 interleaved into slot 0) -----------
                with tc.tile_pool(name="ps_o", bufs=1, space="PSUM") as ps_o:

                    gm = 0
                    gp = 0
                    post_queue = []

                    def post_slot(s, pso, rsacc, last=False):
                        psr = ps_p.tile([128, 512], f32, tag="psp", name="psr")
                        nc.tensor.matmul(psr[:, :], ones_full[:], rsacc[:],
                                         start=True, stop=True)
                        rs_sb = rsp.tile([128, 512], bf16, tag="rs_sb",
                                         name="rs_sb")
                        nc.vector.tensor_copy(out=rs_sb[:], in_=psr[:])
                        rs_row2 = rsp.tile([1, 512], bf16, tag="rs_row2",
                                           name="rs_row2")
                        nc.scalar.copy(out=rs_row2[:], in_=psr[0:1, :])
                        psT = ps_p.tile([128, 4, 128], bf16, tag="psp",
                                        name="psT", padded_shape=[128, 4, 256])
                        for t in range(4):
                            nc.tensor.transpose(psT[:, t, :],
                                                rs_sb[:, 128 * t:128 * (t + 1)],
                                                ident_bf[:])
                        rs_t = rsp.tile([128, 4], f32, tag="rs_t", name="rs_t")
                        nc.vector.reciprocal(out=rs_t[:], in_=psT[:, :, 0])
                        po_sb = postp.tile([128, 2, 512], bf16, tag="po_sb",
                                           name="po_sb")
                        nc.scalar.copy(out=po_sb[:, 0, :], in_=pso[:, 0, :])
                        nc.vector.tensor_copy(out=po_sb[:, 1, :],
                                              in_=pso[:, 1, :])
                        obuf = postp.tile([128, 4, E], bf16, tag="obuf",
                                          name="obuf")
                        for t in range(4):
                            pso3 = ps_p.tile([128, E], f32, tag="psp",
                                             name="pso3",
                                             padded_shape=[128, 512])
                            for eh in range(2):
                                nc.tensor.matmul(
                                    pso3[:], po_sb[:, eh, 128 * t:128 * (t + 1)],
                                    w_all[:, 2, eh, :], start=(eh == 0), stop=False,
                                    skip_group_check=True)
                            nc.tensor.matmul(
                                pso3[:], rs_row2[0:1, 128 * t:128 * (t + 1)],
                                bv_sb[:], start=False, stop=True,
                                skip_group_check=True)
                            nc.vector.tensor_scalar_mul(obuf[:, t, :], pso3[:],
                                                        rs_t[:, t:t + 1])
                            if last:
                                nc.sync.dma_start(
                                    out=out_ext[512 * s + 128 * t:
                                                512 * s + 128 * (t + 1), :],
                                    in_=obuf[:, t, :])
                        if not last:
                            nc.sync.dma_start(
                                out=out_ext[512 * s:512 * (s + 1), :].rearrange(
                                    "(t p) e -> p t e", p=128),
                                in_=obuf[:])

                    def emit_scores(s, p, npair):
                        nonlocal gm, gp
                        masked = p >= npair - NMASK
                        pp = p - (npair - NMASK)
                        pss = ps_s.tile([128, 2, 512], f32, tag="pss",
                                        name="pss")
                        pT = pTp.tile([128, 2, 512], bf16, tag="pT", name="pT")
                        starts = []
                        for i in range(2):
                            w = 2 * pp + i
                            st = max(0, (w - 4) * 128) if masked else 0
                            starts.append(st)
                            ll = 2 * p + i
                            for fh in range(2):
                                nc.tensor.matmul(
                                    pss[:, i, st:],
                                    kT2[ll // 4][:, fh,
                                                 128 * (ll % 4):128 * (ll % 4 + 1)],
                                    qT2[s][:, fh, st:],
                                    start=(fh == 0),
                                    stop=(fh == 1) and not masked,
                                    skip_group_check=masked)
                            if masked:
                                # the causal band of window tile w is always
                                # [128*(w%4), 128*(w%4)+128); below-band cols
                                # are memset (w>=5) or blocked for the other
                                # parity (w<4, table rows cover [0, band_hi));
                                # pad pairs are killed by ebias=-1e30.
                                lo = 0 if w < 4 else 128 * (w % 4)
                                hi = 128 * (w % 4) + 128
                                nc.tensor.matmul(
                                    pss[:, i, lo:hi], ident_big[:],
                                    maskt[:, gm, i, lo:hi],
                                    start=False, stop=True,
                                    skip_group_check=True)
                            if st > 0:
                                nc.vector.memset(pT[:, i, 0:st], 0.0)
                        if masked:
                            gm += 1
                        bias_ap = ebias[:, gp:gp + 1]
                        gp += 1
                        if starts[0] == 0 and starts[1] == 0:
                            nc.scalar.activation(out=pT[:], in_=pss[:],
                                                 func=Exp, bias=bias_ap,
                                                 scale=1.0)
                        else:
                            for i in range(2):
                                nc.scalar.activation(
                                    out=pT[:, i, starts[i]:],
                                    in_=pss[:, i, starts[i]:],
                                    func=Exp, bias=bias_ap, scale=1.0)
                        return pT

                    def emit_pv(s, p, npair, pso, rsacc, pT):
                        masked = p >= npair - NMASK
                        pp = p - (npair - NMASK)
                        for i in range(2):
                            st = max(0, (2 * pp + i - 4) * 128) if masked else 0
                            ll = 2 * p + i
                            for eh in range(2):
                                nc.tensor.matmul(
                                    pso[:, eh, st:],
                                    z_nat[ll // 4][:, ll % 4,
                                                   128 * eh:128 * (eh + 1)],
                                    pT[:, i, st:],
                                    start=(p == 0 and i == 0),
                                    stop=(p == npair - 1 and i == 1),
                                    skip_group_check=True)
                        tmp = rsp.tile([128, 512], bf16, tag="rtmp",
                                       name="rtmp")
                        nc.vector.tensor_tensor(out=tmp[:], in0=pT[:, 0, :],
                                                in1=pT[:, 1, :], op=Add)
                        # tail of the chain on the faster DVE so the psr
                        # matmul of post_slot unblocks sooner
                        acc_eng = nc.vector if p >= npair - 3 else nc.gpsimd
                        if p == 0:
                            nc.gpsimd.tensor_copy(out=rsacc[:], in_=tmp[:])
                        else:
                            acc_eng.tensor_tensor(out=rsacc[:], in0=rsacc[:],
                                                  in1=tmp[:], op=Add)

                    pending = []

                    # -- slot 0 with phase-1 interleave --
                    kproj(0)
                    qproj(0)
                    zdone, xdone = 1, 1
                    npair = PADN[0] // 2
                    pso = ps_o.tile([128, 2, 512], f32, tag="pso",
                                    name="pso")
                    rsacc = rsp.tile([128, 512], f32, tag="racc",
                                     name="racc")
                    for p in range(npair):
                        while zdone < 8 and 4 * zdone < 2 * p + 2:
                            kproj(zdone)
                            if xdone < 4:
                                qproj(xdone)
                                xdone += 1
                            zdone += 1
                        pT = emit_scores(0, p, npair)
                        pending.append((0, p, npair, pso, rsacc, pT))
                        if len(pending) > 2:
                            emit_pv(*pending.pop(0))
                    while zdone < 8:
                        kproj(zdone)
                        zdone += 1
                    while xdone < 4:
                        qproj(xdone)
                        xdone += 1
                    post_queue.append((0, pso, rsacc))

                    # -- slots 1..3 --
                    with tc.tile_pool(name="ps_p", bufs=2, space="PSUM") as ps_p:
                        for s in range(1, NSLOT):
                            npair = PADN[s] // 2
                            pso = ps_o.tile([128, 2, 512], f32, tag="pso",
                                            name="pso")
                            rsacc = rsp.tile([128, 512], f32, tag="racc",
                                             name="racc")
                            for p in range(npair):
                                pT = emit_scores(s, p, npair)
                                pending.append((s, p, npair, pso, rsacc, pT))
                                if p == 2 and post_queue:
                                    post_slot(*post_queue.pop())
                                if len(pending) > 2:
                                    emit_pv(*pending.pop(0))
                            post_queue.append((s, pso, rsacc))
                        while pending:
                            emit_pv(*pending.pop(0))
                        post_slot(*post_queue.pop(), last=True)

    nc.compile()
    return nc


def _get_nc():
    if "nc" not in _COMPILED:
        _COMPILED["nc"] = _build()
    return _COMPILED["nc"]


def _make_masks():
    """Blocked-region tables per parity: [16 pairs, 128 k, 2, 512 q] in {0,1},
    1 = BLOCKED (gets -1e9 added to the score)."""
    import ml_dtypes
    fp8 = ml_dtypes.float8_e4m3
    ky = np.arange(128)[:, None]
    x = np.arange(512)[None, :]
    diag = [((x < 128 * t + ky)).astype(np.float32) for t in range(4)]
    keepall = np.zeros((128, 512), np.float32)
    blockall = np.ones((128, 512), np.float32)
    res = []
    for par in range(2):
        tiles = []
        for s in range(NSLOT):
            valid = 4 * (CHUNKS[par][s] + 1)
            padded = PADN[s]
            for ll in range(padded - 8, padded):
                if ll >= valid:
                    tiles.append(blockall)   # pad tile
                elif ll >= valid - 4:
                    tiles.append(diag[ll - (valid - 4)])
                else:
                    tiles.append(keepall)
        m = np.stack(tiles).reshape(16, 2, 128, 512).transpose(2, 0, 1, 3)
        res.append(np.ascontiguousarray(m.astype(fp8)))
    return res


def _make_ebias():
    """Exp bias per pair: 0 valid, -1e30 pad; [2][40] f32."""
    res = []
    for par in range(2):
        vals = []
        for s in range(NSLOT):
            valid = 4 * (CHUNKS[par][s] + 1)
            for p in range(PADN[s] // 2):
                vals.append(0.0 if 2 * p < valid else -1e30)
        res.append(np.asarray(vals, dtype=np.float32))
    return res


def kernel(X, Z, mask, Wq, bq, Wk, bk, Wv, bv):
    import ml_dtypes
    bf16 = ml_dtypes.bfloat16
    X = np.asarray(X, dtype=np.float32)
    Z = np.asarray(Z, dtype=np.float32)
    mask_np = np.asarray(mask)

    causal = bool(np.array_equal(
        mask_np != 0, np.tril(np.ones((S, S), dtype=bool))))
    if not causal:
        return _numpy_ref(X, Z, mask_np, Wq, bq, Wk, bk, Wv, bv)

    from concourse.bass_utils import run_bass_kernel_spmd

    nc = _get_nc()

    def w2(W):
        # [128, 2, 256]: [p, h, f] = W[f, 128h+p]
        return np.ascontiguousarray(
            np.asarray(W, np.float32).T.reshape(2, 128, 256)
            .transpose(1, 0, 2).astype(bf16))

    wall = np.ascontiguousarray(
        np.stack([w2(Wq), w2(Wk), w2(Wv)]).transpose(1, 0, 2, 3))
    bqs = np.ascontiguousarray(
        (np.asarray(bq, np.float32) / 16.0).reshape(2, 128).T)
    bvr = np.ascontiguousarray(np.asarray(bv, dtype=np.float32).astype(bf16))
    masks = _make_masks()
    ebias = _make_ebias()
    # znat [128, 32, E]: [p, ll, e] = Z[128*ll + p, e]
    zb = [np.ascontiguousarray(
        Z[b].reshape(32, 128, E).transpose(1, 0, 2).astype(bf16))
        for b in range(B)]
    # zt [128, 8, 2, 512]: [p, c, h, k'] = Z[512*c + k', 128*h + p]
    ztb = [np.ascontiguousarray(
        Z[b].T.reshape(2, 128, 8, 512).transpose(1, 2, 0, 3).astype(bf16))
        for b in range(B)]

    in_maps = []
    for c in range(8):
        b, par = c // 2, c % 2
        xb = X[b].reshape(8, 512, E)
        x_shard = xb[list(CHUNKS[par])].reshape(2048, E)
        # xt [128, 4, 2, 512]: [p, s, h, q'] = x_shard[512*s + q', 128*h + p]
        xt = np.ascontiguousarray(
            x_shard.T.reshape(2, 128, 4, 512).transpose(1, 2, 0, 3)
            .astype(bf16))
        in_maps.append({
            "znat": zb[b],
            "zt": ztb[b],
            "xt": xt,
            "wall": wall,
            "bqs": bqs, "bvr": bvr,
            "masks": masks[par], "ebias": ebias[par],
        })

    res = run_bass_kernel_spmd(nc, in_maps, core_ids=list(range(8)))

    out = np.empty((B, S, E), dtype=np.float32)
    for c in range(8):
        b, par = c // 2, c % 2
        o = res.results[c]["out"].astype(np.float32).reshape(NSLOT, 512, E)
        for s in range(NSLOT):
            ch = CHUNKS[par][s]
            out[b, 512 * ch:512 * (ch + 1)] = o[s]
    return out


def _numpy_ref(X, Z, mask, Wq, bq, Wk, bk, Wv, bv):
    q = np.einsum("bse,fe->bsf", X, Wq) + bq
    k = np.einsum("bse,fe->bsf", Z, Wk) + bk
    v = np.einsum("bse,fe->bsf", Z, Wv) + bv
    s = np.einsum("bqe,bke->bqk", q, k) / np.sqrt(np.float32(X.shape[-1]))
    s = np.where(mask == 0, -np.inf, s)
    s = s - s.max(axis=-1, keepdims=True)
    p = np.exp(s)
    p /= p.sum(axis=-1, keepdims=True)
    return np.einsum("bqk,bke->bqe", p, v).astype(np.float32)


# revision 51
# speedup vs baseline: 1.0124x; 1.0124x over previous
"""Trainium2 Bass kernel v6: batched causal attention (B=4, S=4096, E=256, f32).

Sharding: 2 cores per batch element; QUERY chunks split within the pair
(even core gets 512-row chunks {7,5,2,0}, odd {6,4,3,1}) so causal work is
perfectly balanced with NO cross-core communication.  Both cores hold full
K/V for their batch.  SPMD-uniform instruction stream: 4 slots with padded
k-tile counts (32,24,16,8); per-core DATA (mask table) kills padding tiles
and applies causal masks.

v6 key points:
  - all inputs shipped bf16; Z is shipped BOTH natural (PV stationary) and
    pre-TRANSPOSED (projection moving operand); X only pre-transposed.
    This removes all 96 PE transposes + their PSUM round-trips.
  - output written bf16, host upcasts.
  - PE warm-up matmuls ramp the clock while the first chunks stream in.
  - causal masking additive on the PE (-1e9 ident x blocked-pattern fp8
    moving), fused into the score accumulation group.
  - score/mask matmuls + exp skip fully-blocked column prefixes in the
    last-8 k-tile window (start = max(0,(w-4)*128), parity-uniform).

Per-core dataflow (bf16 matmuls, f32 PSUM):
  phase1: Q^T=(WqT@X^T + bq)/16, K^T=WkT@Z^T directly from DMA'd
  transposed chunks, interleaved into slot 0's attention pairs.
  attention: per k-tile pair, S^T = K^T(stat).Q^T (+mask matmul on diag);
  exp on scalar -> P^T bf16; O'^T += Z(stat)@P^T (V projection deferred);
  rowsum via DVE pair-sums + gpsimd accumulation (chain tail on DVE so the
  per-slot reduce unblocks sooner).
  post (per slot, overlapped): rowsum reduced by ones-matmul, reciprocal
  on DVE, O = O'@Wv^T + bv*rowsum (rank-1), scaled by 1/rowsum, bf16 out
  (last slot: per-128-row output DMAs to shorten the tail).

Measured on 8xTRN2: ~119.4us HW exec (baseline v3: 144-168us), rel err
6.2e-3 vs the f32 reference (gate 2e-2).
"""

import numpy as np

B = 4
S = 4096
E = 256
NSLOT = 4
PADN = (32, 24, 16, 8)       # padded k-tiles per slot
CHUNKS = ((7, 5, 2, 0), (6, 4, 3, 1))   # slot -> 512-chunk, per parity
NMASK = 4                    # masked pairs per slot (last 4)

_COMPILED = {}


def _build():
    import concourse.bass as bass
    import concourse.tile as tile
    from concourse import mybir, bacc
    from concourse.masks import make_identity

    f32 = mybir.dt.float32
    bf16 = mybir.dt.bfloat16
    fp8 = mybir.dt.float8e4
    Exp = mybir.ActivationFunctionType.Exp
    Add = mybir.AluOpType.add

    nc = bacc.Bacc("TRN2", target_bir_lowering=False, debug=False,
                   enable_asserts=True, num_devices=8)

    # all big inputs are pre-arranged host-side into partition-major layouts
    # so every DMA is identity-mapped: 128 descriptor rows, cheap to issue.
    z_ext = nc.dram_tensor("znat", [128, 32, E], bf16, kind="ExternalInput")
    zt_ext = nc.dram_tensor("zt", [128, 8, 2, 512], bf16,
                            kind="ExternalInput")
    xt_ext = nc.dram_tensor("xt", [128, 4, 2, 512], bf16,
                            kind="ExternalInput")
    w_ext = nc.dram_tensor("wall", [128, 3, 2, 256], bf16,
                           kind="ExternalInput")
    bqs_ext = nc.dram_tensor("bqs", [128, 42], f32, kind="ExternalInput")
    bv_ext = nc.dram_tensor("bvr", [E], bf16, kind="ExternalInput")
    masks_ext = nc.dram_tensor("masks", [128, 16, 2, 512], fp8,
                               kind="ExternalInput")
    out_ext = nc.dram_tensor("out", [2048, E], bf16, kind="ExternalOutput")

    with tile.TileContext(nc) as tc:
        with tc.tile_pool(name="singles", bufs=1) as singles:
            ident_bf = singles.tile([128, 128], bf16)
            make_identity(nc, ident_bf[:])

            # ---- big persistent SBUF (per-chunk tiles so DMAs overlap) -----
            z_nat = [singles.tile([128, 4, E], bf16, name=f"z_nat{i}")
                     for i in range(8)]
            zT = [singles.tile([128, 2, 512], bf16, name=f"zT{i}")
                  for i in range(8)]
            xT = [singles.tile([128, 2, 512], bf16, name=f"xT{i}")
                  for i in range(4)]
            kT2 = [singles.tile([128, 2, 512], bf16, name=f"kT2_{i}")
                   for i in range(8)]
            qT2 = [singles.tile([128, 2, 512], bf16, name=f"qT2_{i}")
                   for i in range(4)]
            maskt = singles.tile([128, 16, 2, 512], fp8)
            w_all = singles.tile([128, 3, 2, 256], bf16, name="w_all")

            # ---- per-chunk identity-layout input DMAs, spread over the
            # three DMA issue paths (sync+scalar HWDGE, gpsimd SWDGE).
            # Rings are FIFO at ~1/3 of the 358GB/s core budget each, so
            # order strictly by first-use time; tiny tensors go first. ------
            # bqs cols 0:2 = bq/16 halves; cols 2:42 = per-pair exp bias
            bqs = singles.tile([128, 42], f32)
            nc.sync.dma_start(out=bqs[:], in_=bqs_ext[:])
            nc.scalar.dma_start(out=w_all[:], in_=w_ext.ap())

            for _c in range(8):
                eng = nc.sync if _c % 2 == 0 else nc.gpsimd
                eng.dma_start(out=zT[_c][:], in_=zt_ext[:, _c, :, :])
                eng.dma_start(out=z_nat[_c][:],
                              in_=z_ext[:, 4 * _c:4 * (_c + 1), :])
                if _c < 4:
                    nc.scalar.dma_start(out=xT[_c][:],
                                        in_=xt_ext[:, _c, :, :])
            # masks ride at the tail of the sync/gpsimd rings: first needed
            # at pair 12 (gm<8) / pair 32 (gm>=8), long after the chunks.
            nc.sync.dma_start(out=maskt[:, 0:8, :, :],
                              in_=masks_ext[:, 0:8, :, :])
            nc.gpsimd.dma_start(out=maskt[:, 8:16, :, :],
                                in_=masks_ext[:, 8:16, :, :])

            # ---- PE warm-up: dummy matmuls ramp the clock to full p-state
            # while the first chunks stream in -------------------------------
            warm = singles.tile([128, 512], bf16)
            nc.vector.memset(warm[:], 0.0)
            with tc.tile_pool(name="ps_w", bufs=1, space="PSUM") as ps_w:
                psw = ps_w.tile([128, 512], f32)
                for _ in range(13):
                    nc.tensor.matmul(psw[:], warm[:, 0:128], warm[:],
                                     start=True, stop=True)

            ident_big = singles.tile([128, 128], bf16)
            nc.vector.tensor_scalar_mul(ident_big[:], ident_bf[:], -1e9)
            bv_sb = singles.tile([1, E], bf16)
            nc.scalar.dma_start(out=bv_sb[:], in_=bv_ext.ap().rearrange(
                "(one e) -> one e", one=1))
            ones_full = singles.tile([128, 128], f32)
            nc.vector.memset(ones_full[:], 1.0)

            with tc.tile_pool(name="ps_s", bufs=2, space="PSUM") as ps_s, \
                 tc.tile_pool(name="pTp", bufs=6) as pTp, \
                 tc.tile_pool(name="rsp", bufs=4) as rsp, \
                 tc.tile_pool(name="postp", bufs=2) as postp:

                # ---- phase 1: projections from pre-transposed chunks -------
                def kproj(sc):
                    psk = ps_s.tile([128, 2, 512], f32, tag="pss", name="psk")
                    for ft in range(2):
                        for eh in range(2):
                            nc.tensor.matmul(
                                psk[:, ft, :],
                                w_all[:, 1, eh, 128 * ft:128 * (ft + 1)],
                                zT[sc][:, eh, :],
                                start=(eh == 0), stop=(eh == 1))
                    nc.vector.tensor_copy(out=kT2[sc][:], in_=psk[:])

                def qproj(s):
                    psq = ps_s.tile([128, 2, 512], f32, tag="pss", name="psq")
                    for ft in range(2):
                        for eh in range(2):
                            nc.tensor.matmul(
                                psq[:, ft, :],
                                w_all[:, 0, eh, 128 * ft:128 * (ft + 1)],
                                xT[s][:, eh, :],
                                start=(eh == 0), stop=(eh == 1))
                    for ft in range(2):
                        nc.vector.tensor_scalar(
                            out=qT2[s][:, ft, :],
                            in0=psq[:, ft, :],
                            scalar1=1.0 / 16.0, scalar2=bqs[:, ft:ft + 1],
                            op0=mybir.AluOpType.mult, op1=Add)

                # ---- attention (phase 1 interleaved into slot 0) -----------
                with tc.tile_pool(name="ps_o", bufs=1, space="PSUM") as ps_o:

                    gm = 0
                    gp = 0
                    post_queue = []

                    def post_slot(s, pso, rsacc, last=False):
                        psr = ps_p.tile([128, 512], f32, tag="psp", name="psr")
                        nc.tensor.matmul(psr[:, :], ones_full[:], rsacc[:],
                                         start=True, stop=True)
                        rs_sb = rsp.tile([128, 512], bf16, tag="rs_sb",
                                         name="rs_sb")
                        nc.vector.tensor_copy(out=rs_sb[:], in_=psr[:])
                        rs_row2 = rsp.tile([1, 512], bf16, tag="rs_row2",
                                           name="rs_row2")
                        nc.scalar.copy(out=rs_row2[:], in_=psr[0:1, :])
                        psT = ps_p.tile([128, 4, 128], bf16, tag="psp",
                                        name="psT", padded_shape=[128, 4, 256])
                        for t in range(4):
                            nc.tensor.transpose(psT[:, t, :],
                                                rs_sb[:, 128 * t:128 * (t + 1)],
                                                ident_bf[:])
                        rs_t = rsp.tile([128, 4], f32, tag="rs_t", name="rs_t")
                        nc.vector.reciprocal(out=rs_t[:], in_=psT[:, :, 0])
                        po_sb = postp.tile([128, 2, 512], bf16, tag="po_sb",
                                           name="po_sb")
                        nc.scalar.copy(out=po_sb[:, 0, :], in_=pso[:, 0, :])
                        nc.vector.tensor_copy(out=po_sb[:, 1, :],
                                              in_=pso[:, 1, :])
                        obuf = postp.tile([128, 4, E], bf16, tag="obuf",
                                          name="obuf")
                        for t in range(4):
                            pso3 = ps_p.tile([128, E], f32, tag="psp",
                                             name="pso3",
                                             padded_shape=[128, 512])
                            for eh in range(2):
                                nc.tensor.matmul(
                                    pso3[:], po_sb[:, eh, 128 * t:128 * (t + 1)],
                                    w_all[:, 2, eh, :], start=(eh == 0), stop=False,
                                    skip_group_check=True)
                            nc.tensor.matmul(
                                pso3[:], rs_row2[0:1, 128 * t:128 * (t + 1)],
                                bv_sb[:], start=False, stop=True,
                                skip_group_check=True)
                            nc.vector.tensor_scalar_mul(obuf[:, t, :], pso3[:],
                                                        rs_t[:, t:t + 1])
                            if last:
                                nc.sync.dma_start(
                                    out=out_ext[512 * s + 128 * t:
                                                512 * s + 128 * (t + 1), :],
                                    in_=obuf[:, t, :])
                        if not last:
                            nc.sync.dma_start(
                                out=out_ext[512 * s:512 * (s + 1), :].rearrange(
                                    "(t p) e -> p t e", p=128),
                                in_=obuf[:])

                    def emit_scores(s, p, npair):
                        nonlocal gm, gp
                        masked = p >= npair - NMASK
                        pp = p - (npair - NMASK)
                        pss = ps_s.tile([128, 2, 512], f32, tag="pss",
                                        name="pss")
                        pT = pTp.tile([128, 2, 512], bf16, tag="pT", name="pT")
                        starts = []
                        for i in range(2):
                            w = 2 * pp + i
                            st = max(0, (w - 4) * 128) if masked else 0
                            starts.append(st)
                            ll = 2 * p + i
                            for fh in range(2):
                                nc.tensor.matmul(
                                    pss[:, i, st:],
                                    kT2[ll // 4][:, fh,
                                                 128 * (ll % 4):128 * (ll % 4 + 1)],
                                    qT2[s][:, fh, st:],
                                    start=(fh == 0),
                                    stop=(fh == 1) and not masked,
                                    skip_group_check=masked)
                            if masked:
                                # the causal band of window tile w is always
                                # [128*(w%4), 128*(w%4)+128); below-band cols
                                # are memset (w>=5) or blocked for the other
                                # parity (w<4, table rows cover [0, band_hi));
                                # pad pairs are killed by ebias=-1e30.
                                lo = 0 if w < 4 else 128 * (w % 4)
                                hi = 128 * (w % 4) + 128
                                nc.tensor.matmul(
                                    pss[:, i, lo:hi], ident_big[:],
                                    maskt[:, gm, i, lo:hi],
                                    start=False, stop=True,
                                    skip_group_check=True)
                            if st > 0:
                                nc.vector.memset(pT[:, i, 0:st], 0.0)
                        if masked:
                            gm += 1
                        bias_ap = bqs[:, 2 + gp:3 + gp]
                        gp += 1
                        if starts[0] == 0 and starts[1] == 0:
                            nc.scalar.activation(out=pT[:], in_=pss[:],
                                                 func=Exp, bias=bias_ap,
                                                 scale=1.0)
                        else:
                            for i in range(2):
                                nc.scalar.activation(
                                    out=pT[:, i, starts[i]:],
                                    in_=pss[:, i, starts[i]:],
                                    func=Exp, bias=bias_ap, scale=1.0)
                        return pT

                    def emit_pv(s, p, npair, pso, rsacc, pT):
                        masked = p >= npair - NMASK
                        pp = p - (npair - NMASK)
                        for i in range(2):
                            st = max(0, (2 * pp + i - 4) * 128) if masked else 0
                            ll = 2 * p + i
                            for eh in range(2):
                                nc.tensor.matmul(
                                    pso[:, eh, st:],
                                    z_nat[ll // 4][:, ll % 4,
                                                   128 * eh:128 * (eh + 1)],
                                    pT[:, i, st:],
                                    start=(p == 0 and i == 0),
                                    stop=(p == npair - 1 and i == 1),
                                    skip_group_check=True)
                        tmp = rsp.tile([128, 512], bf16, tag="rtmp",
                                       name="rtmp")
                        nc.vector.tensor_tensor(out=tmp[:], in0=pT[:, 0, :],
                                                in1=pT[:, 1, :], op=Add)
                        # tail of the chain on the faster DVE so the psr
                        # matmul of post_slot unblocks sooner
                        acc_eng = nc.vector if p >= npair - 3 else nc.gpsimd
                        if p == 0:
                            nc.gpsimd.tensor_copy(out=rsacc[:], in_=tmp[:])
                        else:
                            acc_eng.tensor_tensor(out=rsacc[:], in0=rsacc[:],
                                                  in1=tmp[:], op=Add)

                    pending = []

                    # -- slot 0 with phase-1 interleave --
                    kproj(0)
                    qproj(0)
                    zdone, xdone = 1, 1
                    npair = PADN[0] // 2
                    pso = ps_o.tile([128, 2, 512], f32, tag="pso",
                                    name="pso")
                    rsacc = rsp.tile([128, 512], f32, tag="racc",
                                     name="racc")
                    for p in range(npair):
                        while zdone < 8 and 4 * zdone < 2 * p + 2:
                            kproj(zdone)
                            if xdone < 4:
                                qproj(xdone)
                                xdone += 1
                            zdone += 1
                        pT = emit_scores(0, p, npair)
                        pending.append((0, p, npair, pso, rsacc, pT))
                        if len(pending) > 2:
                            emit_pv(*pending.pop(0))
                    while zdone < 8:
                        kproj(zdone)
                        zdone += 1
                    while xdone < 4:
                        qproj(xdone)
                        xdone += 1
                    post_queue.append((0, pso, rsacc))

                    # -- slots 1..3 --
                    with tc.tile_pool(name="ps_p", bufs=2, space="PSUM") as ps_p:
                        for s in range(1, NSLOT):
                            npair = PADN[s] // 2
                            pso = ps_o.tile([128, 2, 512], f32, tag="pso",
                                            name="pso")
                            rsacc = rsp.tile([128, 512], f32, tag="racc",
                                             name="racc")
                            for p in range(npair):
                                pT = emit_scores(s, p, npair)
                                pending.append((s, p, npair, pso, rsacc, pT))
                                if p == 2 and post_queue:
                                    post_slot(*post_queue.pop())
                                if len(pending) > 2:
                                    emit_pv(*pending.pop(0))
                            post_queue.append((s, pso, rsacc))
                        while pending:
                            emit_pv(*pending.pop(0))
                        post_slot(*post_queue.pop(), last=True)

    nc.compile()
    return nc


def _get_nc():
    if "nc" not in _COMPILED:
        _COMPILED["nc"] = _build()
    return _COMPILED["nc"]


def _make_masks():
    """Blocked-region tables per parity: [16 pairs, 128 k, 2, 512 q] in {0,1},
    1 = BLOCKED (gets -1e9 added to the score)."""
    import ml_dtypes
    fp8 = ml_dtypes.float8_e4m3
    ky = np.arange(128)[:, None]
    x = np.arange(512)[None, :]
    diag = [((x < 128 * t + ky)).astype(np.float32) for t in range(4)]
    keepall = np.zeros((128, 512), np.float32)
    blockall = np.ones((128, 512), np.float32)
    res = []
    for par in range(2):
        tiles = []
        for s in range(NSLOT):
            valid = 4 * (CHUNKS[par][s] + 1)
            padded = PADN[s]
            for ll in range(padded - 8, padded):
                if ll >= valid:
                    tiles.append(blockall)   # pad tile
                elif ll >= valid - 4:
                    tiles.append(diag[ll - (valid - 4)])
                else:
                    tiles.append(keepall)
        m = np.stack(tiles).reshape(16, 2, 128, 512).transpose(2, 0, 1, 3)
        res.append(np.ascontiguousarray(m.astype(fp8)))
    return res


def _make_ebias():
    """Exp bias per pair: 0 valid, -1e30 pad; [2][40] f32."""
    res = []
    for par in range(2):
        vals = []
        for s in range(NSLOT):
            valid = 4 * (CHUNKS[par][s] + 1)
            for p in range(PADN[s] // 2):
                vals.append(0.0 if 2 * p < valid else -1e30)
        res.append(np.asarray(vals, dtype=np.float32))
    return res


def kernel(X, Z, mask, Wq, bq, Wk, bk, Wv, bv):
    import ml_dtypes
    bf16 = ml_dtypes.bfloat16
    X = np.asarray(X, dtype=np.float32)
    Z = np.asarray(Z, dtype=np.float32)
    mask_np = np.asarray(mask)

    causal = bool(np.array_equal(
        mask_np != 0, np.tril(np.ones((S, S), dtype=bool))))
    if not causal:
        return _numpy_ref(X, Z, mask_np, Wq, bq, Wk, bk, Wv, bv)

    from concourse.bass_utils import run_bass_kernel_spmd

    nc = _get_nc()

    def w2(W):
        # [128, 2, 256]: [p, h, f] = W[f, 128h+p]
        return np.ascontiguousarray(
            np.asarray(W, np.float32).T.reshape(2, 128, 256)
            .transpose(1, 0, 2).astype(bf16))

    wall = np.ascontiguousarray(
        np.stack([w2(Wq), w2(Wk), w2(Wv)]).transpose(1, 0, 2, 3))
    bqs2 = (np.asarray(bq, np.float32) / 16.0).reshape(2, 128).T
    bvr = np.ascontiguousarray(np.asarray(bv, dtype=np.float32).astype(bf16))
    masks = _make_masks()
    ebias = _make_ebias()
    bqse = [np.ascontiguousarray(np.concatenate(
        [bqs2, np.broadcast_to(ebias[par], (128, 40))], axis=1,
        dtype=np.float32)) for par in range(2)]
    # znat [128, 32, E]: [p, ll, e] = Z[128*ll + p, e]
    zb = [np.ascontiguousarray(
        Z[b].reshape(32, 128, E).transpose(1, 0, 2).astype(bf16))
        for b in range(B)]
    # zt [128, 8, 2, 512]: [p, c, h, k'] = Z[512*c + k', 128*h + p]
    ztb = [np.ascontiguousarray(
        Z[b].T.reshape(2, 128, 8, 512).transpose(1, 2, 0, 3).astype(bf16))
        for b in range(B)]

    in_maps = []
    for c in range(8):
        b, par = c // 2, c % 2
        xb = X[b].reshape(8, 512, E)
        x_shard = xb[list(CHUNKS[par])].reshape(2048, E)
        # xt [128, 4, 2, 512]: [p, s, h, q'] = x_shard[512*s + q', 128*h + p]
        xt = np.ascontiguousarray(
            x_shard.T.reshape(2, 128, 4, 512).transpose(1, 2, 0, 3)
            .astype(bf16))
        in_maps.append({
            "znat": zb[b],
            "zt": ztb[b],
            "xt": xt,
            "wall": wall,
            "bqs": bqse[par], "bvr": bvr,
            "masks": masks[par],
        })

    res = run_bass_kernel_spmd(nc, in_maps, core_ids=list(range(8)))

    out = np.empty((B, S, E), dtype=np.float32)
    for c in range(8):
        b, par = c // 2, c % 2
        o = res.results[c]["out"].astype(np.float32).reshape(NSLOT, 512, E)
        for s in range(NSLOT):
            ch = CHUNKS[par][s]
            out[b, 512 * ch:512 * (ch + 1)] = o[s]
    return out


def _numpy_ref(X, Z, mask, Wq, bq, Wk, bk, Wv, bv):
    q = np.einsum("bse,fe->bsf", X, Wq) + bq
    k = np.einsum("bse,fe->bsf", Z, Wk) + bk
    v = np.einsum("bse,fe->bsf", Z, Wv) + bv
    s = np.einsum("bqe,bke->bqk", q, k) / np.sqrt(np.float32(X.shape[-1]))
    s = np.where(mask == 0, -np.inf, s)
    s = s - s.max(axis=-1, keepdims=True)
    p = np.exp(s)
    p /= p.sum(axis=-1, keepdims=True)
    return np.einsum("bqk,bke->bqe", p, v).astype(np.float32)


# revision 52
# speedup vs baseline: 1.0345x; 1.0218x over previous
"""Trainium2 Bass kernel v6: batched causal attention (B=4, S=4096, E=256, f32).

Sharding: 2 cores per batch element; QUERY chunks split within the pair
(even core gets 512-row chunks {7,5,2,0}, odd {6,4,3,1}) so causal work is
perfectly balanced with NO cross-core communication.  Both cores hold full
K/V for their batch.  SPMD-uniform instruction stream: 4 slots with padded
k-tile counts (32,24,16,8); per-core DATA (mask table) kills padding tiles
and applies causal masks.

v6 key points:
  - all inputs shipped bf16; Z is shipped BOTH natural (PV stationary) and
    pre-TRANSPOSED (projection moving operand); X only pre-transposed.
    This removes all 96 PE transposes + their PSUM round-trips.
  - output written bf16, host upcasts.
  - PE warm-up matmuls ramp the clock while the first chunks stream in.
  - causal masking additive on the PE (-1e9 ident x blocked-pattern fp8
    moving), fused into the score accumulation group.
  - score/mask matmuls + exp skip fully-blocked column prefixes in the
    last-8 k-tile window (start = max(0,(w-4)*128), parity-uniform).

Per-core dataflow (bf16 matmuls, f32 PSUM):
  phase1: Q^T=(WqT@X^T + bq)/16, K^T=WkT@Z^T directly from DMA'd
  transposed chunks, interleaved into slot 0's attention pairs.
  attention: per k-tile pair, S^T = K^T(stat).Q^T (+mask matmul on diag);
  exp on scalar -> P^T bf16; O'^T += Z(stat)@P^T (V projection deferred);
  rowsum via DVE pair-sums + gpsimd accumulation (chain tail on DVE so the
  per-slot reduce unblocks sooner).
  post (per slot, overlapped): rowsum reduced by ones-matmul, reciprocal
  on DVE, O = O'@Wv^T + bv*rowsum (rank-1), scaled by 1/rowsum, bf16 out
  (last slot: per-128-row output DMAs to shorten the tail).

Measured on 8xTRN2: ~119.4us HW exec (baseline v3: 144-168us), rel err
6.2e-3 vs the f32 reference (gate 2e-2).
"""

import numpy as np

B = 4
S = 4096
E = 256
NSLOT = 4
PADN = (32, 24, 16, 8)       # padded k-tiles per slot
CHUNKS = ((7, 5, 2, 0), (6, 4, 3, 1))   # slot -> 512-chunk, per parity
NMASK = 4                    # masked pairs per slot (last 4)

_COMPILED = {}


def _build():
    import concourse.bass as bass
    import concourse.tile as tile
    from concourse import mybir, bacc
    from concourse.masks import make_identity

    f32 = mybir.dt.float32
    bf16 = mybir.dt.bfloat16
    fp8 = mybir.dt.float8e4
    Exp = mybir.ActivationFunctionType.Exp
    Add = mybir.AluOpType.add

    nc = bacc.Bacc("TRN2", target_bir_lowering=False, debug=False,
                   enable_asserts=True, num_devices=8)

    # all big inputs are pre-arranged host-side into partition-major layouts
    # so every DMA is identity-mapped: 128 descriptor rows, cheap to issue.
    z_ext = nc.dram_tensor("znat", [128, 32, E], bf16, kind="ExternalInput")
    zt_ext = nc.dram_tensor("zt", [128, 8, 2, 512], bf16,
                            kind="ExternalInput")
    xt_ext = nc.dram_tensor("xt", [128, 4, 2, 512], bf16,
                            kind="ExternalInput")
    w_ext = nc.dram_tensor("wall", [128, 3, 2, 256], bf16,
                           kind="ExternalInput")
    bqs_ext = nc.dram_tensor("bqs", [128, 2], f32, kind="ExternalInput")
    bv_ext = nc.dram_tensor("bvr", [E], bf16, kind="ExternalInput")
    masks_ext = nc.dram_tensor("masks", [128, 16, 2, 512], fp8,
                               kind="ExternalInput")
    out_ext = nc.dram_tensor("out", [2048, E], bf16, kind="ExternalOutput")

    with tile.TileContext(nc) as tc:
        with tc.tile_pool(name="singles", bufs=1) as singles:
            ident_bf = singles.tile([128, 128], bf16)
            make_identity(nc, ident_bf[:])

            # ---- big persistent SBUF (per-chunk tiles so DMAs overlap) -----
            z_nat = [singles.tile([128, 4, E], bf16, name=f"z_nat{i}")
                     for i in range(8)]
            zT = [singles.tile([128, 2, 512], bf16, name=f"zT{i}")
                  for i in range(8)]
            xT = [singles.tile([128, 2, 512], bf16, name=f"xT{i}")
                  for i in range(4)]
            kT2 = [singles.tile([128, 2, 512], bf16, name=f"kT2_{i}")
                   for i in range(8)]
            qT2 = [singles.tile([128, 2, 512], bf16, name=f"qT2_{i}")
                   for i in range(4)]
            maskt = singles.tile([128, 16, 2, 512], fp8)
            w_all = singles.tile([128, 3, 2, 256], bf16, name="w_all")

            # ---- per-chunk identity-layout input DMAs, spread over the
            # three DMA issue paths (sync+scalar HWDGE, gpsimd SWDGE).
            # Rings are FIFO at ~1/3 of the 358GB/s core budget each, so
            # order strictly by first-use time; tiny tensors go first. ------
            bqs = singles.tile([128, 2], f32)
            nc.sync.dma_start(out=bqs[:], in_=bqs_ext[:])
            nc.scalar.dma_start(out=w_all[:], in_=w_ext.ap())

            for _c in range(8):
                eng = nc.sync if _c % 2 == 0 else nc.gpsimd
                eng.dma_start(out=zT[_c][:], in_=zt_ext[:, _c, :, :])
                eng.dma_start(out=z_nat[_c][:],
                              in_=z_ext[:, 4 * _c:4 * (_c + 1), :])
                if _c < 4:
                    nc.scalar.dma_start(out=xT[_c][:],
                                        in_=xt_ext[:, _c, :, :])
            # masks ride at the tail of the sync/gpsimd rings: first needed
            # at pair 12 (gm<8) / pair 32 (gm>=8), long after the chunks.
            nc.sync.dma_start(out=maskt[:, 0:8, :, :],
                              in_=masks_ext[:, 0:8, :, :])
            nc.gpsimd.dma_start(out=maskt[:, 8:16, :, :],
                                in_=masks_ext[:, 8:16, :, :])

            # ---- PE warm-up: dummy matmuls ramp the clock to full p-state
            # while the first chunks stream in -------------------------------
            warm = singles.tile([128, 512], bf16)
            nc.vector.memset(warm[:], 0.0)
            with tc.tile_pool(name="ps_w", bufs=1, space="PSUM") as ps_w:
                psw = ps_w.tile([128, 512], f32)
                for _ in range(13):
                    nc.tensor.matmul(psw[:], warm[:, 0:128], warm[:],
                                     start=True, stop=True)

            ident_big = singles.tile([128, 128], bf16)
            nc.vector.tensor_scalar_mul(ident_big[:], ident_bf[:], -1e9)
            bv_sb = singles.tile([1, E], bf16)
            nc.scalar.dma_start(out=bv_sb[:], in_=bv_ext.ap().rearrange(
                "(one e) -> one e", one=1))
            ones_full = singles.tile([128, 128], f32)
            nc.vector.memset(ones_full[:], 1.0)

            with tc.tile_pool(name="ps_s", bufs=2, space="PSUM") as ps_s, \
                 tc.tile_pool(name="pTp", bufs=6) as pTp, \
                 tc.tile_pool(name="rsp", bufs=4) as rsp, \
                 tc.tile_pool(name="postp", bufs=2) as postp:

                # ---- phase 1: projections from pre-transposed chunks -------
                def kproj(sc):
                    psk = ps_s.tile([128, 2, 512], f32, tag="pss", name="psk")
                    for ft in range(2):
                        for eh in range(2):
                            nc.tensor.matmul(
                                psk[:, ft, :],
                                w_all[:, 1, eh, 128 * ft:128 * (ft + 1)],
                                zT[sc][:, eh, :],
                                start=(eh == 0), stop=(eh == 1))
                    nc.vector.tensor_copy(out=kT2[sc][:], in_=psk[:])

                def qproj(s):
                    psq = ps_s.tile([128, 2, 512], f32, tag="pss", name="psq")
                    for ft in range(2):
                        for eh in range(2):
                            nc.tensor.matmul(
                                psq[:, ft, :],
                                w_all[:, 0, eh, 128 * ft:128 * (ft + 1)],
                                xT[s][:, eh, :],
                                start=(eh == 0), stop=(eh == 1))
                    for ft in range(2):
                        nc.vector.tensor_scalar(
                            out=qT2[s][:, ft, :],
                            in0=psq[:, ft, :],
                            scalar1=1.0 / 16.0, scalar2=bqs[:, ft:ft + 1],
                            op0=mybir.AluOpType.mult, op1=Add)

                # ---- attention (phase 1 interleaved into slot 0) -----------
                with tc.tile_pool(name="ps_o", bufs=1, space="PSUM") as ps_o:

                    gm = 0
                    post_queue = []

                    def post_slot(s, pso, rsacc, last=False):
                        psr = ps_p.tile([128, 512], f32, tag="psp", name="psr")
                        nc.tensor.matmul(psr[:, :], ones_full[:], rsacc[:],
                                         start=True, stop=True)
                        rs_sb = rsp.tile([128, 512], bf16, tag="rs_sb",
                                         name="rs_sb")
                        nc.vector.tensor_copy(out=rs_sb[:], in_=psr[:])
                        rs_row2 = rsp.tile([1, 512], bf16, tag="rs_row2",
                                           name="rs_row2")
                        nc.scalar.copy(out=rs_row2[:], in_=psr[0:1, :])
                        psT = ps_p.tile([128, 4, 128], bf16, tag="psp",
                                        name="psT", padded_shape=[128, 4, 256])
                        for t in range(4):
                            nc.tensor.transpose(psT[:, t, :],
                                                rs_sb[:, 128 * t:128 * (t + 1)],
                                                ident_bf[:])
                        rs_t = rsp.tile([128, 4], f32, tag="rs_t", name="rs_t")
                        nc.vector.reciprocal(out=rs_t[:], in_=psT[:, :, 0])
                        po_sb = postp.tile([128, 2, 512], bf16, tag="po_sb",
                                           name="po_sb")
                        nc.scalar.copy(out=po_sb[:, 0, :], in_=pso[:, 0, :])
                        nc.vector.tensor_copy(out=po_sb[:, 1, :],
                                              in_=pso[:, 1, :])
                        obuf = postp.tile([128, 4, E], bf16, tag="obuf",
                                          name="obuf")
                        for t in range(4):
                            pso3 = ps_p.tile([128, E], f32, tag="psp",
                                             name="pso3",
                                             padded_shape=[128, 512])
                            for eh in range(2):
                                nc.tensor.matmul(
                                    pso3[:], po_sb[:, eh, 128 * t:128 * (t + 1)],
                                    w_all[:, 2, eh, :], start=(eh == 0), stop=False,
                                    skip_group_check=True)
                            nc.tensor.matmul(
                                pso3[:], rs_row2[0:1, 128 * t:128 * (t + 1)],
                                bv_sb[:], start=False, stop=True,
                                skip_group_check=True)
                            nc.vector.tensor_scalar_mul(obuf[:, t, :], pso3[:],
                                                        rs_t[:, t:t + 1])
                            if last:
                                nc.sync.dma_start(
                                    out=out_ext[512 * s + 128 * t:
                                                512 * s + 128 * (t + 1), :],
                                    in_=obuf[:, t, :])
                        if not last:
                            nc.sync.dma_start(
                                out=out_ext[512 * s:512 * (s + 1), :].rearrange(
                                    "(t p) e -> p t e", p=128),
                                in_=obuf[:])

                    def emit_scores(s, p, npair):
                        nonlocal gm
                        masked = p >= npair - NMASK
                        pp = p - (npair - NMASK)
                        pss = ps_s.tile([128, 2, 512], f32, tag="pss",
                                        name="pss")
                        pT = pTp.tile([128, 2, 512], bf16, tag="pT", name="pT")
                        starts = []
                        for i in range(2):
                            st = max(0, (2 * pp + i - 4) * 128) if masked else 0
                            starts.append(st)
                            ll = 2 * p + i
                            for fh in range(2):
                                nc.tensor.matmul(
                                    pss[:, i, st:],
                                    kT2[ll // 4][:, fh,
                                                 128 * (ll % 4):128 * (ll % 4 + 1)],
                                    qT2[s][:, fh, st:],
                                    start=(fh == 0),
                                    stop=(fh == 1) and not masked)
                            if masked:
                                nc.tensor.matmul(
                                    pss[:, i, st:], ident_big[:],
                                    maskt[:, gm, i, st:],
                                    start=False, stop=True)
                            if st > 0:
                                nc.vector.memset(pT[:, i, 0:st], 0.0)
                        if masked:
                            gm += 1
                        if starts[0] == 0 and starts[1] == 0:
                            nc.scalar.activation(out=pT[:], in_=pss[:],
                                                 func=Exp, bias=0.0, scale=1.0)
                        else:
                            for i in range(2):
                                nc.scalar.activation(
                                    out=pT[:, i, starts[i]:],
                                    in_=pss[:, i, starts[i]:],
                                    func=Exp, bias=0.0, scale=1.0)
                        return pT

                    def emit_pv(s, p, npair, pso, rsacc, pT):
                        masked = p >= npair - NMASK
                        pp = p - (npair - NMASK)
                        for i in range(2):
                            st = max(0, (2 * pp + i - 4) * 128) if masked else 0
                            ll = 2 * p + i
                            for eh in range(2):
                                nc.tensor.matmul(
                                    pso[:, eh, st:],
                                    z_nat[ll // 4][:, ll % 4,
                                                   128 * eh:128 * (eh + 1)],
                                    pT[:, i, st:],
                                    start=(p == 0 and i == 0),
                                    stop=(p == npair - 1 and i == 1),
                                    skip_group_check=True)
                        tmp = rsp.tile([128, 512], bf16, tag="rtmp",
                                       name="rtmp")
                        nc.vector.tensor_tensor(out=tmp[:], in0=pT[:, 0, :],
                                                in1=pT[:, 1, :], op=Add)
                        # tail of the chain on the faster DVE so the psr
                        # matmul of post_slot unblocks sooner
                        acc_eng = nc.vector if p >= npair - 3 else nc.gpsimd
                        if p == 0:
                            nc.gpsimd.tensor_copy(out=rsacc[:], in_=tmp[:])
                        else:
                            acc_eng.tensor_tensor(out=rsacc[:], in0=rsacc[:],
                                                  in1=tmp[:], op=Add)

                    pending = []

                    # -- slot 0 with phase-1 interleave --
                    kproj(0)
                    qproj(0)
                    zdone, xdone = 1, 1
                    npair = PADN[0] // 2
                    pso = ps_o.tile([128, 2, 512], f32, tag="pso",
                                    name="pso")
                    rsacc = rsp.tile([128, 512], f32, tag="racc",
                                     name="racc")
                    for p in range(npair):
                        while zdone < 8 and 4 * zdone < 2 * p + 2:
                            kproj(zdone)
                            if xdone < 4:
                                qproj(xdone)
                                xdone += 1
                            zdone += 1
                        pT = emit_scores(0, p, npair)
                        pending.append((0, p, npair, pso, rsacc, pT))
                        if len(pending) > 2:
                            emit_pv(*pending.pop(0))
                    while zdone < 8:
                        kproj(zdone)
                        zdone += 1
                    while xdone < 4:
                        qproj(xdone)
                        xdone += 1
                    post_queue.append((0, pso, rsacc))

                    # -- slots 1..3 --
                    with tc.tile_pool(name="ps_p", bufs=2, space="PSUM") as ps_p:
                        for s in range(1, NSLOT):
                            npair = PADN[s] // 2
                            pso = ps_o.tile([128, 2, 512], f32, tag="pso",
                                            name="pso")
                            rsacc = rsp.tile([128, 512], f32, tag="racc",
                                             name="racc")
                            for p in range(npair):
                                pT = emit_scores(s, p, npair)
                                pending.append((s, p, npair, pso, rsacc, pT))
                                if len(pending) > 2:
                                    emit_pv(*pending.pop(0))
                                if p == 1 and post_queue:
                                    post_slot(*post_queue.pop())
                            post_queue.append((s, pso, rsacc))
                        while pending:
                            emit_pv(*pending.pop(0))
                        post_slot(*post_queue.pop(), last=True)

    nc.compile()
    return nc


def _get_nc():
    if "nc" not in _COMPILED:
        _COMPILED["nc"] = _build()
    return _COMPILED["nc"]


def _make_masks():
    """Blocked-region tables per parity: [16 pairs, 128 k, 2, 512 q] in {0,1},
    1 = BLOCKED (gets -1e9 added to the score)."""
    import ml_dtypes
    fp8 = ml_dtypes.float8_e4m3
    ky = np.arange(128)[:, None]
    x = np.arange(512)[None, :]
    diag = [((x < 128 * t + ky)).astype(np.float32) for t in range(4)]
    keepall = np.zeros((128, 512), np.float32)
    blockall = np.ones((128, 512), np.float32)
    res = []
    for par in range(2):
        tiles = []
        for s in range(NSLOT):
            valid = 4 * (CHUNKS[par][s] + 1)
            padded = PADN[s]
            for ll in range(padded - 8, padded):
                if ll >= valid:
                    tiles.append(blockall)   # pad tile
                elif ll >= valid - 4:
                    tiles.append(diag[ll - (valid - 4)])
                else:
                    tiles.append(keepall)
        m = np.stack(tiles).reshape(16, 2, 128, 512).transpose(2, 0, 1, 3)
        res.append(np.ascontiguousarray(m.astype(fp8)))
    return res


def kernel(X, Z, mask, Wq, bq, Wk, bk, Wv, bv):
    import ml_dtypes
    bf16 = ml_dtypes.bfloat16
    X = np.asarray(X, dtype=np.float32)
    Z = np.asarray(Z, dtype=np.float32)
    mask_np = np.asarray(mask)

    causal = bool(np.array_equal(
        mask_np != 0, np.tril(np.ones((S, S), dtype=bool))))
    if not causal:
        return _numpy_ref(X, Z, mask_np, Wq, bq, Wk, bk, Wv, bv)

    from concourse.bass_utils import run_bass_kernel_spmd

    nc = _get_nc()

    def w2(W):
        # [128, 2, 256]: [p, h, f] = W[f, 128h+p]
        return np.ascontiguousarray(
            np.asarray(W, np.float32).T.reshape(2, 128, 256)
            .transpose(1, 0, 2).astype(bf16))

    wall = np.ascontiguousarray(
        np.stack([w2(Wq), w2(Wk), w2(Wv)]).transpose(1, 0, 2, 3))
    bqs = np.ascontiguousarray(
        (np.asarray(bq, np.float32) / 16.0).reshape(2, 128).T)
    bvr = np.ascontiguousarray(np.asarray(bv, dtype=np.float32).astype(bf16))
    masks = _make_masks()
    # znat [128, 32, E]: [p, ll, e] = Z[128*ll + p, e]
    zb = [np.ascontiguousarray(
        Z[b].reshape(32, 128, E).transpose(1, 0, 2).astype(bf16))
        for b in range(B)]
    # zt [128, 8, 2, 512]: [p, c, h, k'] = Z[512*c + k', 128*h + p]
    ztb = [np.ascontiguousarray(
        Z[b].T.reshape(2, 128, 8, 512).transpose(1, 2, 0, 3).astype(bf16))
        for b in range(B)]

    in_maps = []
    for c in range(8):
        b, par = c // 2, c % 2
        xb = X[b].reshape(8, 512, E)
        x_shard = xb[list(CHUNKS[par])].reshape(2048, E)
        # xt [128, 4, 2, 512]: [p, s, h, q'] = x_shard[512*s + q', 128*h + p]
        xt = np.ascontiguousarray(
            x_shard.T.reshape(2, 128, 4, 512).transpose(1, 2, 0, 3)
            .astype(bf16))
        in_maps.append({
            "znat": zb[b],
            "zt": ztb[b],
            "xt": xt,
            "wall": wall,
            "bqs": bqs, "bvr": bvr,
            "masks": masks[par],
        })

    res = run_bass_kernel_spmd(nc, in_maps, core_ids=list(range(8)))

    out = np.empty((B, S, E), dtype=np.float32)
    for c in range(8):
        b, par = c // 2, c % 2
        o = res.results[c]["out"].astype(np.float32).reshape(NSLOT, 512, E)
        for s in range(NSLOT):
            ch = CHUNKS[par][s]
            out[b, 512 * ch:512 * (ch + 1)] = o[s]
    return out


def _numpy_ref(X, Z, mask, Wq, bq, Wk, bk, Wv, bv):
    q = np.einsum("bse,fe->bsf", X, Wq) + bq
    k = np.einsum("bse,fe->bsf", Z, Wk) + bk
    v = np.einsum("bse,fe->bsf", Z, Wv) + bv
    s = np.einsum("bqe,bke->bqk", q, k) / np.sqrt(np.float32(X.shape[-1]))
    s = np.where(mask == 0, -np.inf, s)
    s = s - s.max(axis=-1, keepdims=True)
    p = np.exp(s)
    p /= p.sum(axis=-1, keepdims=True)
    return np.einsum("bqk,bke->bqe", p, v).astype(np.float32)
